# revision 4
# baseline (speedup 1.0000x reference)
"""Trainium2 Bass kernel for CapsuleLayer (dynamic routing, ROUTINGS=3).

Strategy: shard J=2048 across 8 cores (J_local=256). All heavy
O(B*K*J*Di*Do) contractions run on-device as PE matmuls:
  - s-einsum launches: routing coeffs c folded into x on host
    (y = c*x), device contracts (j,i) per k:  s[b,k,o] = y_k @ W_k.
  - logit-update launches: block-diagonal x packing computes
    u_hat tiles on PE, then contracts o with v (replicated on host)
    via vector engine mul+reduce: db[b,k,j] = sum_o u_hat*v.
Host does only tiny glue between launches: softmax over K, squash,
and summing per-core partials (the J all-reduce).
"""
import numpy as np

B, J, DI = 64, 2048, 16
K, DO = 32, 32
NC_ = 8
JL = J // NC_          # 256 j per core
NJG = JL // 8          # 32 groups of 8 j
NBS = B // 16          # 4 batch sub-chunks of 16
NT = JL * DI // 128    # 32 contraction tiles of 128 for s-einsum
EPS = 1e-7

_cache = {}


def _squash(s):
    s2 = np.sum(s * s, axis=-1, keepdims=True)
    return (s2 / (1.0 + s2) / np.sqrt(s2 + EPS)) * s


def _softmax_k(b):
    m = b.max(axis=1, keepdims=True)
    e = np.exp(b - m)
    return e / e.sum(axis=1, keepdims=True)


def _build_programs():
    import concourse.bacc as bacc
    import concourse.tile as tile
    import concourse.mybir as mybir

    bf16 = mybir.dt.bfloat16
    f32 = mybir.dt.float32

    # ---- S program: s_part[k,b,o] = sum_(j,i) y[k,(j,i),b] * w[k,(j,i),o]
    nc_s = bacc.Bacc("TRN2", target_bir_lowering=False, debug=False,
                     num_devices=NC_)
    Y_d = nc_s.dram_tensor("Y", [K, NT, 128, B], bf16, kind="ExternalInput")
    WR_d = nc_s.dram_tensor("WR", [K, NT, 128, DO], bf16, kind="ExternalInput")
    SP_d = nc_s.dram_tensor("SP", [K, B, DO], f32, kind="ExternalOutput")
    with tile.TileContext(nc_s) as tc:
        with tc.tile_pool(name="yp", bufs=3) as yp, \
             tc.tile_pool(name="wp", bufs=3) as wp, \
             tc.tile_pool(name="ps", bufs=1, space="PSUM") as ps:
            for k in range(K):
                yt = yp.tile([128, NT * B], bf16, tag="y")
                wt = wp.tile([128, NT * DO], bf16, tag="w")
                for t in range(NT):
                    nc_s.sync.dma_start(yt[:, t * B:(t + 1) * B],
                                        Y_d.ap()[k, t])
                    nc_s.sync.dma_start(wt[:, t * DO:(t + 1) * DO],
                                        WR_d.ap()[k, t])
                acc = ps.tile([B, DO], f32, tag="acc")
                for t in range(NT):
                    nc_s.tensor.matmul(
                        acc[:], yt[:, t * B:(t + 1) * B],
                        wt[:, t * DO:(t + 1) * DO],
                        start=(t == 0), stop=(t == NT - 1))
                accs = yp.tile([B, DO], f32, tag="accs")
                nc_s.vector.tensor_copy(accs[:], acc[:])
                nc_s.sync.dma_start(SP_d.ap()[k], accs[:])
    nc_s.compile()

    # ---- D program: db[p=(jj,bb),k] per (bs,jg) = sum_o uhat*vrep
    nc_d = bacc.Bacc("TRN2", target_bir_lowering=False, debug=False,
                     num_devices=NC_)
    XB_d = nc_d.dram_tensor("XB", [NBS, NJG, 128, 128], bf16,
                            kind="ExternalInput")
    W2_d = nc_d.dram_tensor("W2", [NJG, 128, K * DO], bf16,
                            kind="ExternalInput")
    VR_d = nc_d.dram_tensor("VR", [NBS, 128, K * DO], f32,
                            kind="ExternalInput")
    DB_d = nc_d.dram_tensor("DB", [NBS, NJG, 128, K], f32,
                            kind="ExternalOutput")
    with tile.TileContext(nc_d) as tc:
        with tc.tile_pool(name="xp", bufs=3) as xp, \
             tc.tile_pool(name="w2p", bufs=3) as w2p, \
             tc.tile_pool(name="vp", bufs=1) as vp, \
             tc.tile_pool(name="pr", bufs=3) as prp, \
             tc.tile_pool(name="dbp", bufs=3) as dbp, \
             tc.tile_pool(name="ps", bufs=3, space="PSUM") as ps:
            vts = []
            for bs in range(NBS):
                vt = vp.tile([128, K * DO], f32, tag=f"v{bs}")
                nc_d.sync.dma_start(vt[:], VR_d.ap()[bs])
                vts.append(vt)
            for jg in range(NJG):
                w2t = w2p.tile([128, K * DO], bf16, tag="w2")
                nc_d.sync.dma_start(w2t[:], W2_d.ap()[jg])
                for bs in range(NBS):
                    xt = xp.tile([128, 128], bf16, tag="x")
                    nc_d.sync.dma_start(xt[:], XB_d.ap()[bs, jg])
                    um = ps.tile([128, K * DO], f32, tag="um")
                    for h in range(2):
                        nc_d.tensor.matmul(
                            um[:, h * 512:(h + 1) * 512], xt[:],
                            w2t[:, h * 512:(h + 1) * 512],
                            start=True, stop=True)
                    pr = prp.tile([128, K * DO], f32, tag="pr")
                    nc_d.vector.tensor_mul(pr[:], um[:], vts[bs][:])
                    db = dbp.tile([128, K], f32, tag="db")
                    nc_d.vector.tensor_reduce(
                        db[:], pr[:].rearrange("p (k o) -> p k o", o=DO),
                        axis=mybir.AxisListType.X, op=mybir.AluOpType.add)
                    nc_d.sync.dma_start(DB_d.ap()[bs, jg], db[:])
    nc_d.compile()
    return nc_s, nc_d


def kernel(inputs, W):
    import ml_dtypes
    from concourse import bass_utils
    bf = ml_dtypes.bfloat16
    x = np.asarray(inputs, np.float32)
    Wf = np.asarray(W, np.float32)

    if "progs" not in _cache:
        _cache["progs"] = _build_programs()
    nc_s, nc_d = _cache["progs"]

    # per-core host-side constant operands
    xs, WRs, W2s, XBs = [], [], [], []
    for c in range(NC_):
        xl = x[:, c * JL:(c + 1) * JL, :]            # [B, JL, DI]
        Wl = Wf[c * JL:(c + 1) * JL]                 # [JL, K, DI, DO]
        xs.append(xl)
        # WR[k,t,(jj,i),o] ; t covers 8 j
        WRs.append(np.ascontiguousarray(
            Wl.transpose(1, 0, 2, 3).reshape(K, NT, 128, DO).astype(bf)))
        # W2[jg,(jj,i),(k,o)]
        W2s.append(np.ascontiguousarray(
            Wl.reshape(NJG, 8, K, DI, DO).transpose(0, 1, 3, 2, 4)
            .reshape(NJG, 128, K * DO).astype(bf)))
        # XB block-diag: [bs,jg,(jj,i),(jj,bb)]
        xr = xl.reshape(NBS, 16, NJG, 8, DI)          # bs,bb,jg,jj,i
        xb = np.zeros((NBS, NJG, 8, DI, 8, 16), np.float32)
        jj = np.arange(8)
        xb[:, :, jj, :, jj, :] = xr.transpose(3, 0, 2, 4, 1)
        XBs.append(xb.reshape(NBS, NJG, 128, 128).astype(bf))

    def run_s(c_route):
        maps = []
        for c in range(NC_):
            cl = c_route[:, :, c * JL:(c + 1) * JL]   # [B,K,JL]
            y = cl[:, :, :, None] * xs[c][:, None, :, :]   # [B,K,JL,DI]
            Y = (y.transpose(1, 2, 3, 0)                   # k,j,i,b
                 .reshape(K, NT, 128, B).astype(bf))
            maps.append({"Y": np.ascontiguousarray(Y), "WR": WRs[c]})
        res = bass_utils.run_bass_kernel_spmd(
            nc_s, maps, core_ids=list(range(NC_)))
        sp = sum(np.asarray(r["SP"], np.float32) for r in res.results)
        return np.ascontiguousarray(sp.transpose(1, 0, 2))  # [B,K,DO]

    def run_d(v):
        vr = v.reshape(NBS, 16, K * DO).astype(np.float32)
        maps = []
        for c in range(NC_):
            VR = np.ascontiguousarray(np.tile(vr, (1, 8, 1)))
            maps.append({"XB": XBs[c], "W2": W2s[c], "VR": VR})
        res = bass_utils.run_bass_kernel_spmd(
            nc_d, maps, core_ids=list(range(NC_)))
        db = np.empty((B, K, J), np.float32)
        for c in range(NC_):
            d = np.asarray(res.results[c]["DB"], np.float32)
            d = d.reshape(NBS, NJG, 8, 16, K)          # bs,jg,jj,bb,k
            d = d.transpose(0, 3, 4, 1, 2).reshape(B, K, JL)
            db[:, :, c * JL:(c + 1) * JL] = d
        return db

    c0 = np.full((B, K, J), 1.0 / K, np.float32)
    v = _squash(run_s(c0))
    b = run_d(v)
    v = _squash(run_s(_softmax_k(b)))
    b = b + run_d(v)
    v = _squash(run_s(_softmax_k(b)))
    return v.astype(np.float32)
